# revision 1
# baseline (speedup 1.0000x reference)
"""DFMConv2d Trainium2 kernel.

Reference computation (per sample b):
  pooled = mean_{h,w} x[b]                          [C=256]
  h      = relu(pooled @ w1.T + b1)                 [128]
  mix    = softmax((h @ w2.T + b2).reshape(256, 8)) [256, 8]
  y      = conv3x3_SAME(x[b], base_filters)         [8, 64, 64]
  out[b] = einsum('on,nhw->ohw', mix, y)            [256, 64, 64]

Strategy (8 NeuronCores, data-parallel over batch, 8 samples/core):
  - x cast to bf16 on host (halves input DMA; conv error ~2.4e-3 rel).
  - conv: 9 shifted/trimmed accumulating matmuls per (h-chunk, c-chunk),
    K=128(c), M=8(n), col-group packed 4 samples via tile_position=(0,32jj).
  - attention MLP + softmax on-chip (fp32), mixT built via PE transpose +
    SBUF partition-shift DMAs.
  - mix: K=8 matmuls row-group packed via tile_position=(32jj,0), fp32.
  - out copied PSUM->SBUF (ACT/DVE) and DMA'd out as fp32.
"""
import sys

sys.path.insert(0, "/opt/trn_rl_repo")

import numpy as np
import ml_dtypes

import concourse.bacc as bacc
import concourse.tile as tile
import concourse.mybir as mybir
from concourse.bass_utils import run_bass_kernel_spmd
from contextlib import ExitStack

F32 = mybir.dt.float32
BF16 = mybir.dt.bfloat16
AFT = mybir.ActivationFunctionType
AXX = mybir.AxisListType.X
ALU = mybir.AluOpType

N_CORES = 8
BPC = 8            # samples per core
C = 256
CO = 256
H = W = 64
HW = H * W
NB = 8             # n_base
HID = 128
NG = 2             # sample groups per core (4 samples each)
GS = 4             # group size
NHC = 8            # h-chunks (8 rows of output each)
CCH = 2            # channel chunks of 128

# taps ordered center-first so the first matmul covers the full psum region
TAPS = [(1, 1)] + [(dy, dx) for dy in range(3) for dx in range(3) if not (dy == 1 and dx == 1)]

_BUILT = None


def _build():
    nc = bacc.Bacc("TRN2", target_bir_lowering=False)

    d_x = nc.dram_tensor("x", [BPC, C, HW], BF16, kind="ExternalInput")
    d_w1t = nc.dram_tensor("w1t", [C, HID], F32, kind="ExternalInput")
    d_b1 = nc.dram_tensor("b1", [HID, 1], F32, kind="ExternalInput")
    d_w2p = nc.dram_tensor("w2p", [HID, NB, CO], F32, kind="ExternalInput")
    d_b2t = nc.dram_tensor("b2t", [128, 2, NB], F32, kind="ExternalInput")
    d_ft = nc.dram_tensor("ft", [128, CCH, 9, NB], BF16, kind="ExternalInput")
    d_id = nc.dram_tensor("ident", [128, 128], F32, kind="ExternalInput")
    d_out = nc.dram_tensor("out", [BPC, 2, 128, HW], F32, kind="ExternalOutput")

    with tile.TileContext(nc) as tc, ExitStack() as ctx:
        prm = ctx.enter_context(tc.tile_pool(name="prm", bufs=1))
        xp = ctx.enter_context(tc.tile_pool(name="xp", bufs=6))
        yp = ctx.enter_context(tc.tile_pool(name="yp", bufs=2))
        op = ctx.enter_context(tc.tile_pool(name="op", bufs=3))
        sm = ctx.enter_context(tc.tile_pool(name="sm", bufs=2))
        ps_c = ctx.enter_context(tc.tile_pool(name="ps_c", bufs=2, space="PSUM"))
        ps_m = ctx.enter_context(tc.tile_pool(name="ps_m", bufs=3, space="PSUM"))
        ps_s = ctx.enter_context(tc.tile_pool(name="ps_s", bufs=1, space="PSUM"))
        ps_l = ctx.enter_context(tc.tile_pool(name="ps_l", bufs=2, space="PSUM"))

        # ---- params (loaded once) ----
        w1t_sb = prm.tile([128, CCH, HID], F32, tag="w1t")
        nc.sync.dma_start(out=w1t_sb, in_=d_w1t[:, :].rearrange("(cc p) h -> p cc h", p=128))
        b1_sb = prm.tile([128, 1], F32, tag="b1")
        nc.sync.dma_start(out=b1_sb, in_=d_b1[:, :])
        w2p_sb = prm.tile([HID, NB, CO], F32, tag="w2p")
        nc.sync.dma_start(out=w2p_sb, in_=d_w2p[:, :, :])
        b2t_sb = prm.tile([128, 2, NB], F32, tag="b2t")
        nc.sync.dma_start(out=b2t_sb, in_=d_b2t[:, :, :])
        ft_sb = prm.tile([128, CCH, 9, NB], BF16, tag="ft")
        nc.sync.dma_start(out=ft_sb, in_=d_ft[:, :, :, :])
        id_sb = prm.tile([128, 128], F32, tag="ident")
        nc.sync.dma_start(out=id_sb, in_=d_id[:, :])

        pooled_sb = prm.tile([128, CCH, BPC], F32, tag="pooled")

        x_t = [None] * BPC
        for g in range(NG):
            # ---- loads + pooling ----
            for jj in range(GS):
                j = GS * g + jj
                xt = xp.tile([128, CCH, HW], BF16, tag="x")
                nc.sync.dma_start(
                    out=xt, in_=d_x[j, :, :].rearrange("(cc p) hw -> p cc hw", p=128))
                x_t[j] = xt
                for cc in range(CCH):
                    nc.vector.reduce_sum(
                        pooled_sb[:, cc, j:j + 1], xt[:, cc, :], axis=AXX)

            # ---- attention MLP for the 4 samples of this group ----
            ph = ps_s.tile([128, GS], F32, tag="h")
            for cc in range(CCH):
                nc.tensor.matmul(ph, w1t_sb[:, cc, :], pooled_sb[:, cc, GS * g:GS * g + GS],
                                 start=(cc == 0), stop=(cc == 1))
            h_sb = sm.tile([128, GS], F32, tag="h")
            nc.scalar.activation(out=h_sb, in_=ph, func=AFT.Relu, bias=b1_sb, scale=1.0)

            mixT_sb = sm.tile([128, 2, 128], F32, tag="mixT")
            for oc in range(2):
                pl = ps_l.tile([128, NB * GS], F32, tag="lg")
                for n in range(NB):
                    nc.tensor.matmul(pl[:, n * GS:(n + 1) * GS],
                                     w2p_sb[:, n, oc * 128:(oc + 1) * 128], h_sb,
                                     start=True, stop=True)
                # logits (+b2) -> sbuf; layout free = (n, b)
                lg_sb = sm.tile([128, NB, GS], F32, tag="lg_sb")
                nc.vector.tensor_tensor(
                    out=lg_sb, in0=pl.rearrange("p (n b) -> p n b", b=GS),
                    in1=bacc.bass.AP(tensor=b2t_sb.tensor,
                                     offset=b2t_sb[:, oc, :].offset,
                                     ap=[b2t_sb.ap[0], [1, NB], [0, GS]]),
                    op=ALU.add)
                ex_sb = sm.tile([128, NB, GS], F32, tag="ex_sb")
                nc.scalar.activation(out=ex_sb, in_=lg_sb, func=AFT.Exp)
                # sum over n (strided inner view), reciprocal
                sums = sm.tile([128, GS], F32, tag="sums")
                nc.vector.reduce_sum(
                    sums,
                    bacc.bass.AP(tensor=ex_sb.tensor, offset=ex_sb.offset,
                                 ap=[ex_sb.ap[0], [1, GS], [GS, NB]]),
                    axis=AXX)
                rec = sm.tile([128, GS], F32, tag="rec")
                nc.vector.reciprocal(rec, sums)
                # normalized mix, written transposed to (b, n) layout
                mix_sb = sm.tile([128, GS, NB], F32, tag="mix_sb")
                nc.vector.tensor_tensor(
                    out=bacc.bass.AP(tensor=mix_sb.tensor, offset=mix_sb.offset,
                                     ap=[mix_sb.ap[0], [1, NB], [NB, GS]]),
                    in0=ex_sb,
                    in1=bacc.bass.AP(tensor=rec.tensor, offset=rec.offset,
                                     ap=[rec.ap[0], [0, NB], [1, GS]]),
                    op=ALU.mult)
                # transpose [128, 32] -> [32, 128]; rows = (b, n)
                ptr = ps_l.tile([GS * NB, 128], F32, tag="lg")
                nc.tensor.transpose(ptr, mix_sb.rearrange("p b n -> p (b n)"), id_sb)
                tr_sb = sm.tile([GS * NB, 128], F32, tag="tr_sb")
                nc.scalar.copy(out=tr_sb, in_=ptr)
                for jj in range(GS):
                    nc.sync.dma_start(out=mixT_sb[32 * jj:32 * jj + NB, oc, :],
                                      in_=tr_sb[NB * jj:NB * jj + NB, :])

            # ---- conv: y[n, hw] per sample, col-group packed ----
            y_sb = yp.tile([128, HW], F32, tag="y")
            for hc in range(NHC):
                yps = ps_c.tile([128, 512], F32, tag="yps")
                n_mm = len(TAPS) * CCH
                for ti, (dy, dx) in enumerate(TAPS):
                    sh, sw = dy - 1, dx - 1
                    r0 = max(8 * hc, -sh) - 8 * hc
                    r1 = min(8 * hc + 8, 64 - max(0, sh)) - 8 * hc
                    w0 = max(0, -sw)
                    w1 = 64 - max(0, sw)
                    for cc in range(CCH):
                        mm_i = ti * CCH + cc
                        for jj in range(GS):
                            j = GS * g + jj
                            outv = yps[32 * jj:32 * jj + NB, :].rearrange(
                                "p (h w) -> p h w", w=64)[:, r0:r1, w0:w1]
                            rhs = x_t[j][:, cc, :].rearrange(
                                "p (h w) -> p h w", w=64)[
                                :, 8 * hc + r0 + sh:8 * hc + r1 + sh, w0 + sw:w1 + sw]
                            nc.tensor.matmul(
                                outv, ft_sb[:, cc, 3 * dy + dx, :], rhs,
                                start=(mm_i == 0), stop=(mm_i == n_mm - 1),
                                tile_position=(0, 32 * jj),
                                skip_group_check=True)
                nc.any.tensor_copy(y_sb[:, 512 * hc:512 * (hc + 1)], yps)

            # ---- mix: out[o, hw] = mixT.T @ y, row-group packed ----
            for oc in range(2):
                for jj in range(GS):
                    j = GS * g + jj
                    ot = op.tile([128, HW], F32, tag="out")
                    for hc in range(NHC):
                        om = ps_m.tile([128, 512], F32, tag="ops")
                        nc.tensor.matmul(
                            om, mixT_sb[32 * jj:32 * jj + NB, oc, :],
                            y_sb[32 * jj:32 * jj + NB, 512 * hc:512 * (hc + 1)],
                            start=True, stop=True, tile_position=(32 * jj, 0))
                        nc.any.tensor_copy(ot[:, 512 * hc:512 * (hc + 1)], om)
                    nc.sync.dma_start(out=d_out[j, oc, :, :], in_=ot)

    nc.compile()
    return nc


def _prep_inputs(x, w1, b1, w2, b2, base_filters):
    """Host-side input layout prep. Returns per-core in_maps."""
    B = x.shape[0]
    xs = np.ascontiguousarray(x.reshape(B, C, HW)).astype(ml_dtypes.bfloat16)
    w1t = np.ascontiguousarray(w1.T).astype(np.float32) / float(HW)
    b1c = np.ascontiguousarray(b1.reshape(HID, 1)).astype(np.float32)
    w2p = np.ascontiguousarray(w2.reshape(CO, NB, HID).transpose(2, 1, 0)).astype(np.float32)
    b2t = np.ascontiguousarray(b2.reshape(2, 128, NB).transpose(1, 0, 2)).astype(np.float32)
    filt = base_filters.reshape(NB, C, 3, 3)
    ft = np.ascontiguousarray(
        filt.reshape(NB, CCH, 128, 9).transpose(2, 1, 3, 0)).astype(ml_dtypes.bfloat16)
    ident = np.eye(128, dtype=np.float32)

    in_maps = []
    for core in range(N_CORES):
        in_maps.append({
            "x": np.ascontiguousarray(xs[core * BPC:(core + 1) * BPC]),
            "w1t": w1t, "b1": b1c, "w2p": w2p, "b2t": b2t,
            "ft": ft, "ident": ident,
        })
    return in_maps


def kernel(x, w1, b1, w2, b2, base_filters):
    global _BUILT
    if _BUILT is None:
        _BUILT = _build()
    nc = _BUILT
    in_maps = _prep_inputs(np.asarray(x, dtype=np.float32),
                           np.asarray(w1, dtype=np.float32),
                           np.asarray(b1, dtype=np.float32),
                           np.asarray(w2, dtype=np.float32),
                           np.asarray(b2, dtype=np.float32),
                           np.asarray(base_filters, dtype=np.float32))
    res = run_bass_kernel_spmd(nc, in_maps, core_ids=list(range(N_CORES)))
    outs = []
    for core in range(N_CORES):
        o = res.results[core]["out"]            # [BPC, 2, 128, HW]
        outs.append(o.reshape(BPC, CO, H, W))
    return np.concatenate(outs, axis=0).astype(np.float32)
